# revision 27
# baseline (speedup 1.0000x reference)
"""Trainium2 Bass kernel for nn_CFTAOBlock2D.

Sharding: pure data-parallel over (batch b, channel-half) -> 8 cores.
Each core gets its batch image with channels PERMUTED so its own 32 output
channels are always [0:32] (SPMD: one program, per-core data differs).

All matmuls run in fp16 (1 cyc/row on PE, ~0.05% quantization); the host
ships x pre-cast to fp16 so every load stays on HWDGE. Residual adds use
exact fp32 x. z is accumulated/stored in fp32; dw taps run on fp16.

Per-core pipeline:
  S1  spectral forward: truncated H-DFT as matmuls (h-layout x) + PE transposes
  S2  mode space: W-DFT, modal multiply (host-precomputed kernel), H-inverse
  S3  per 2048-col f-chunk: channel matmuls (mlp1+gelu, local+dw-dx-taps+mlp2)
      and the W-inverse "y" matmuls, ALL accumulating into one PSUM tile
  S4  z-chain: 6 depthwise taps (DVE/GPSIMD shifted FMA), merges, stats
  S5  instance-norm stats fold (ones-matmul), rstd, affine broadcast
  S6  sweep 2: fused affine+gelu on ACT, residual add, DMA out
"""
from contextlib import ExitStack

import numpy as np

import concourse.bass as bass
import concourse.bacc as bacc
import concourse.tile as tile
from concourse import mybir
from concourse.bass_utils import run_bass_kernel_spmd

F32 = mybir.dt.float32
FP16 = mybir.dt.float16
AX = mybir.AluOpType
AF = mybir.ActivationFunctionType

B, C, H, W = 4, 64, 256, 256
M1, M2 = 32, 32
HALF_M = 16
LSEG, RADIAL_K = 4, 4
LOCAL_SCALE, SPATIAL_SCALE, SPEC_SCALE = 0.3, 0.15, 1.0
HW = H * W            # 65536
NQ, QH = 4, 64        # H quarters
FQ = QH * W           # 16384 packed free per quarter
HALO = 257
NFC = 8               # f-chunks
FC = FQ // NFC        # 2048
OC = 32               # own channels per core
N_CORES = 8

CHAIN_TAPS = [(-1, -1), (-1, 0), (-1, 1), (0, -1), (0, 1), (1, -1), (1, 0), (1, 1)]
CORR_TAPS = [(-1, -1), (-1, 1), (0, -1), (0, 1), (1, -1), (1, 1)]
XQH = 0


# ---------------------------------------------------------------- host math
def _softplus(x):
    x = np.asarray(x, np.float64)
    return np.log1p(np.exp(-np.abs(x))) + np.maximum(x, 0.0)


def _softmax(x):
    e = np.exp(np.asarray(x, np.float64) - np.max(x))
    return e / e.sum()


def _modal_multiplier(f):
    """Combined spectral multiplier M_c: (64, 32, 32) complex128."""
    gh = _softmax(f["seg_h_h"]) * LSEG
    gw = _softmax(f["seg_h_w"]) * LSEG
    seg_r = (np.arange(M1) * LSEG) // M1
    seg_c = (np.arange(M2) * LSEG) // M2
    seg_gain = gh[seg_r][:, None] * gw[seg_c][None, :]

    ky = np.linspace(0.0, 1.0, M1)
    kx = np.linspace(0.0, 1.0, M2)
    Ky, Kx = ky[:, None], kx[None, :]
    r2 = Ky * Ky + Kx * Kx
    r = np.sqrt(r2 + 1e-12)
    nu0 = _softplus(f["nu_log"])
    alpha0 = _softplus(f["alpha_log"])
    c_amp = _softplus(f["c_log"])
    amp_base = np.exp(-nu0 * r2) + c_amp / (1.0 + alpha0 * r2 + 1e-6)
    w0 = (r <= 0.33).astype(np.float64)
    w2b = (r >= 0.66).astype(np.float64)
    w1 = np.maximum(1.0 - w0 - w2b, 0.0)
    g = _softplus(f["band_gain"])
    amp_base = amp_base * ((1.0 + g[0]) * w0 + (1.0 + g[1]) * w1 + (1.0 + g[2]) * w2b)
    phi_base = np.float64(f["omega_y"]) * Ky + np.float64(f["omega_x"]) * Kx

    B_rad = np.stack([r**k for k in range(RADIAL_K)], axis=0)
    amp_delta = np.einsum("ck,khw->chw", _softplus(f["amp_coef"]), B_rad)
    phase_delta = np.einsum(
        "ck,khw->chw", np.asarray(f["phase_coef"], np.float64), B_rad)
    amp_full = amp_base[None] * (1.0 + np.maximum(amp_delta, 0.0))
    phi_full = phi_base[None] + phase_delta
    kernel = (np.cos(phi_full) + 1j * np.sin(phi_full)) * amp_full
    fk = (np.asarray(f["free_kernel_re"], np.float64)
          + 1j * np.asarray(f["free_kernel_im"], np.float64))
    return seg_gain[None] * kernel * SPEC_SCALE * (1.0 + np.float64(f["free_eps"]) * fk)


def _dft_mats():
    hh = np.arange(H)
    fr = np.concatenate([np.arange(HALF_M), np.arange(H - (M1 - HALF_M), H)])
    ang_h = 2.0 * np.pi * np.outer(hh, fr) / H          # (256, 32)
    fhT = np.concatenate([np.cos(ang_h), -np.sin(ang_h)], axis=1)  # (256, 64)

    ww = np.arange(W)
    mm = np.arange(M2)
    ang_w = 2.0 * np.pi * np.outer(ww, mm) / W          # (256, 32)
    FwR, FwI = np.cos(ang_w), -np.sin(ang_w)
    fwA = np.concatenate([FwR, FwI], axis=1)            # (256, 64)
    fwB = np.concatenate([-FwI, FwR], axis=1)           # (256, 64)

    GhR = np.cos(ang_h).T / H                           # (32, 256)
    GhI = np.sin(ang_h).T / H
    ghR2 = np.concatenate([GhR, -GhI], axis=0)          # (64, 256)
    ghI2 = np.concatenate([GhI, GhR], axis=0)           # (64, 256)
    cm = np.full(M2, 2.0)
    cm[0] = 1.0
    GwR = (cm[:, None] * np.cos(ang_w.T)) / W           # (32, 256)
    GwI = (cm[:, None] * np.sin(ang_w.T)) / W
    gw2 = np.concatenate([GwR, -GwI], axis=0)           # (64, 256)
    return fhT, fwA, fwB, ghR2, ghI2, gw2


def _per_core_inputs(inputs):
    f = {k: np.asarray(v) for k, v in inputs.items()}
    x = np.asarray(f["x"], np.float32)
    Mc = _modal_multiplier(f)
    fhT, fwA, fwB, ghR2, ghI2, gw2 = _dft_mats()

    ident = np.eye(128, dtype=np.float32)
    kd = (SPATIAL_SCALE * np.asarray(f["w_dw3"], np.float64)[:, 0])  # (64, 3, 3)
    w_local = np.asarray(f["w_local"], np.float64)
    w_mlp1 = np.asarray(f["w_mlp1"], np.float64)
    w_mlp2 = np.asarray(f["w_mlp2"], np.float64)
    b_local = np.asarray(f["b_local"], np.float64)
    b_dw3 = np.asarray(f["b_dw3"], np.float64)
    b_mlp1 = np.asarray(f["b_mlp1"], np.float64)
    b_mlp2 = np.asarray(f["b_mlp2"], np.float64)
    gamma = np.asarray(f["gamma"], np.float64)
    beta = np.asarray(f["beta"], np.float64)

    qones = np.zeros((128, 32), np.float32)
    for p in range(128):
        qones[p, p % 32] = 1.0
    qonesT = np.ascontiguousarray(qones.T)

    in_maps = []
    for core in range(N_CORES):
        b, half = core // 2, core % 2
        perm = np.concatenate([np.arange(half * 32, half * 32 + 32),
                               np.arange((1 - half) * 32, (1 - half) * 32 + 32)])
        xbv = x[b][perm]
        oc = perm[:OC]

        mcR = np.empty((32, 2, 16, 32), np.float32)
        mcI = np.empty((32, 2, 16, 32), np.float32)
        for ci in range(OC):
            par, pair = ci % 2, ci // 2
            mcR[:, par, pair, :] = Mc[oc[ci]].real.astype(np.float32)
            mcI[:, par, pair, :] = Mc[oc[ci]].imag.astype(np.float32)

        wlocT = (LOCAL_SCALE * w_local[oc][:, perm].T)
        for ci in range(OC):
            wlocT[ci, ci] += kd[oc[ci], 1, 1]       # center dw tap folded in
        wlocT2 = np.zeros((128, 64), np.float64)    # block-diag for q-pairing
        wlocT2[0:64, 0:32] = wlocT
        wlocT2[64:128, 32:64] = wlocT
        wm1T = w_mlp1[:, perm].T
        wm1T2 = np.concatenate([wm1T, wm1T], axis=0)  # (128, 128) dup
        wm2T = w_mlp2[oc].T

        ktaps = np.zeros((NQ, OC, len(CHAIN_TAPS)), np.float32)
        kcorr = np.zeros((NQ, OC, len(CORR_TAPS)), np.float32)
        for t, (dy, dx) in enumerate(CHAIN_TAPS):
            ktaps[:, :, t] = kd[oc, dy + 1, dx + 1][None, :]
        for t, (dy, dx) in enumerate(CORR_TAPS):
            kcorr[:, :, t] = -kd[oc, dy + 1, dx + 1][None, :]

        bconst = (LOCAL_SCALE * b_local[oc] + SPATIAL_SCALE * b_dw3[oc] + b_mlp2[oc])
        bconst128 = np.tile(bconst, NQ).astype(np.float32)[:, None]

        in_maps.append({
            "xbh": np.ascontiguousarray(xbv.astype(np.float16)),
            "xo32": np.ascontiguousarray(xbv[:OC]),
            "fhT": fhT.astype(np.float16),
            "fwA": fwA.astype(np.float16), "fwB": fwB.astype(np.float16),
            "ghR2": ghR2.astype(np.float16), "ghI2": ghI2.astype(np.float16),
            "gw2": gw2.astype(np.float16),
            "mcR": mcR, "mcI": mcI,
            "ident": ident,
            "wlocT2": wlocT2.astype(np.float16),
            "wm1T2": wm1T2.astype(np.float16),
            "wm2T": wm2T.astype(np.float16),
            "ktaps": ktaps.reshape(128, len(CHAIN_TAPS)),
            "kcorr": kcorr.reshape(128, len(CORR_TAPS)),
            "bconst": bconst128,
            "bm1": b_mlp1.astype(np.float32)[:, None],
            "gam": gamma[oc].astype(np.float32)[:, None],
            "bet": beta[oc].astype(np.float32)[:, None],
            "qones": qones, "qonesT": qonesT,
        })
    return in_maps


# ---------------------------------------------------------------- device code
def _build_program():
    nc = bacc.Bacc(None, target_bir_lowering=False, debug=False)
    P = {}

    def di(name, shape, dtype=F32):
        P[name] = nc.declare_dram_parameter(name, list(shape), dtype, isOutput=False)

    di("xbh", (C, H, W), FP16)
    di("xo32", (OC, H, W))
    di("fhT", (256, 64), FP16)
    di("fwA", (256, 64), FP16); di("fwB", (256, 64), FP16)
    di("ghR2", (64, 256), FP16); di("ghI2", (64, 256), FP16)
    di("gw2", (64, 256), FP16)
    di("mcR", (32, 2, 16, 32)); di("mcI", (32, 2, 16, 32))
    di("ident", (128, 128))
    di("wlocT2", (128, 64), FP16); di("wm1T2", (128, 128), FP16)
    di("wm2T", (128, 32), FP16)
    di("ktaps", (128, len(CHAIN_TAPS))); di("kcorr", (128, len(CORR_TAPS)))
    di("bconst", (128, 1)); di("bm1", (128, 1))
    di("gam", (32, 1)); di("bet", (32, 1))
    di("qones", (128, 32)); di("qonesT", (32, 128))
    outp = nc.declare_dram_parameter("outp", [OC, H, W], F32, isOutput=True)

    with tile.TileContext(nc) as tc, ExitStack() as ctx:
        _body(ctx, tc, P, outp)
    nc.finalize()
    return nc


def _body(ctx, tc, P, outp):
    nc = tc.nc
    xbh = P["xbh"]
    xbh_f = xbh.rearrange("c h w -> c (h w)")                    # (64, 65536) fp16
    xoh_f = xbh_f[0:OC, :]                                       # (32, 65536) fp16
    xo_f = P["xo32"].rearrange("c h w -> c (h w)")               # (32, 65536) f32
    out_f = outp.rearrange("c h w -> c (h w)")                   # (32, 65536)

    consts = ctx.enter_context(tc.tile_pool(name="consts", bufs=1))

    def load_const(name, shape, dtype=F32):
        t = consts.tile(list(shape), dtype, tag=name)
        nc.sync.dma_start(out=t, in_=P[name][:])
        return t

    fhT_s = consts.tile([128, 2, 64], FP16, tag="fhT")
    nc.sync.dma_start(out=fhT_s, in_=P["fhT"].rearrange("(t p) m -> p t m", p=128))
    fwA_s = consts.tile([128, 2, 64], FP16, tag="fwA")
    nc.sync.dma_start(out=fwA_s, in_=P["fwA"].rearrange("(t p) m -> p t m", p=128))
    fwB_s = consts.tile([128, 2, 64], FP16, tag="fwB")
    nc.sync.dma_start(out=fwB_s, in_=P["fwB"].rearrange("(t p) m -> p t m", p=128))
    ghR2_s = load_const("ghR2", (64, 256), FP16)
    ghI2_s = load_const("ghI2", (64, 256), FP16)
    gw2_s = load_const("gw2", (64, 256), FP16)
    mcR_s = load_const("mcR", (32, 2, 16, 32))
    mcI_s = load_const("mcI", (32, 2, 16, 32))
    ident_s = load_const("ident", (128, 128))
    wlocT2_s = load_const("wlocT2", (128, 64), FP16)
    wm1T2_s = load_const("wm1T2", (128, 128), FP16)
    wm2T_s = load_const("wm2T", (128, 32), FP16)
    ktaps_s = load_const("ktaps", (128, len(CHAIN_TAPS)))
    kcorr_s = load_const("kcorr", (128, len(CORR_TAPS)))
    bconst_s = load_const("bconst", (128, 1))
    bm1_s = load_const("bm1", (128, 1))
    gam_s = load_const("gam", (32, 1))
    bet_s = load_const("bet", (32, 1))
    qones_s = load_const("qones", (128, 32))
    qonesT_s = load_const("qonesT", (32, 128))

    mid = ctx.enter_context(tc.tile_pool(name="mid", bufs=1))
    # Qst: (64=[QR rm;QI rm], pair, par, wm) ; c = 2*pair+par, linear (c, wm)
    Qst = mid.tile([64, 16, 2, 32], FP16, tag="Qst")
    Qst_c = Qst.rearrange("p a b w -> p (a b) w")                # (64, 32, 32)
    # Zh2: (64=[ZhR wm;ZhI wm], q, c, hl) ; h = q*64+hl
    Zh2 = mid.tile([64, 4, 32, 64], FP16, tag="Zh2")

    # ---------------- S1 + S2: spectral ----------------
    with tc.tile_pool(name="spec1", bufs=1) as sp1:
        T1 = sp1.tile([64, OC, 256], F32, tag="T1")              # (rmRI, c, w)
        T1v = T1.rearrange("p c w -> p (c w)")
        # T1T[wh]: (128 w, RI 2, c 32, rm 32): per-(RI, c-pair) slices contiguous
        T1T0 = sp1.tile([128, 2, OC, 32], FP16, tag="T1T0")
        T1T1 = sp1.tile([128, 2, OC, 32], FP16, tag="T1T1")
        T1T = [T1T0, T1T1]

        with tc.tile_pool(name="xhp", bufs=2) as xhp, \
             tc.tile_pool(name="ps_t1", bufs=1, space="PSUM") as ps_t1, \
             tc.tile_pool(name="ps_tr", bufs=2, space="PSUM") as ps_tr:
            xh = []
            for ht in range(2):
                t = xhp.tile([128, OC, 256], FP16, tag="xh")
                for cb in range(0, OC, 8):
                    nc.sync.dma_start(
                        out=t[:, cb:cb + 8, :],
                        in_=xbh[cb:cb + 8, ht * 128:(ht + 1) * 128, :].rearrange(
                            "c h w -> h c w"))
                xh.append(t)
            for reg in range(4):
                pt = ps_t1.tile([64, 2048], F32, tag="t1p")
                for n in range(4):
                    col = reg * 2048 + n * 512
                    for ht in range(2):
                        nc.tensor.matmul(
                            out=pt[:, n * 512:(n + 1) * 512],
                            lhsT=fhT_s[:, ht, :],
                            rhs=xh[ht].rearrange("p c w -> p (c w)")[:, col:col + 512],
                            start=(ht == 0), stop=(ht == 1))
                nc.vector.tensor_copy(out=T1v[:, reg * 2048:(reg + 1) * 2048], in_=pt)

            # transposes: T1 (64, c, 256w) -> T1T[wh] (128w, RI, c, rm)
            for blk in range(8):
                pt = ps_tr.tile([128, 512], F32, tag="trp")
                for i in range(8):
                    k = blk * 8 + i
                    cch, wh = k // 2, k % 2
                    nc.tensor.transpose(
                        out=pt[:, i * 64:(i + 1) * 64],
                        in_=T1[:, cch, wh * 128:(wh + 1) * 128],
                        identity=ident_s[0:64, 0:64])
                for i in range(8):
                    k = blk * 8 + i
                    cch, wh = k // 2, k % 2
                    nc.scalar.copy(
                        out=T1T[wh][:, :, cch, :],
                        in_=pt[:, i * 64:(i + 1) * 64].rearrange(
                            "p (a b) -> p a b", a=2))

        with tc.tile_pool(name="ps_cp", bufs=1, space="PSUM") as ps_cp, \
             tc.tile_pool(name="ps_zh", bufs=2, space="PSUM") as ps_zh:
            cp = ps_cp.tile([64, 16, 2, 32], F32, tag="cp")      # (rm x par, pair, RI, wm)
            for pr in range(16):
                dst = cp[:, pr, :, :].rearrange("p a b -> p (a b)")
                for wh in range(2):
                    nc.tensor.matmul(out=dst,
                                     lhsT=T1T[wh][:, 0, 2 * pr:2 * pr + 2, :],
                                     rhs=fwA_s[:, wh, :],
                                     start=(wh == 0), stop=False)
                for wh in range(2):
                    nc.tensor.matmul(out=dst,
                                     lhsT=T1T[wh][:, 1, 2 * pr:2 * pr + 2, :],
                                     rhs=fwB_s[:, wh, :],
                                     start=False, stop=(wh == 1))
            tmpA = sp1.tile([32, 16, 32], F32, tag="mtmpA")
            tmpB = sp1.tile([32, 16, 32], F32, tag="mtmpB")
            for par in range(2):
                crs = cp[32 * par:32 * par + 32, :, 0, :]
                cis = cp[32 * par:32 * par + 32, :, 1, :]
                mr = mcR_s[:, par, :, :]
                mi = mcI_s[:, par, :, :]
                nc.vector.tensor_tensor(out=tmpA, in0=crs, in1=mr, op=AX.mult)
                nc.vector.tensor_tensor(out=tmpB, in0=cis, in1=mi, op=AX.mult)
                nc.vector.tensor_tensor(out=Qst[0:32, :, par, :], in0=tmpA, in1=tmpB,
                                        op=AX.subtract)
                nc.vector.tensor_tensor(out=tmpA, in0=cis, in1=mr, op=AX.mult)
                nc.vector.tensor_tensor(out=tmpB, in0=crs, in1=mi, op=AX.mult)
                nc.vector.tensor_tensor(out=Qst[32:64, :, par, :], in0=tmpA, in1=tmpB,
                                        op=AX.add)

            for grp in range(8):                                 # H-inverse, 4 c per region
                zp = ps_zh.tile([64, 4, 256], F32, tag="zhp")
                for i in range(4):
                    cch = grp * 4 + i
                    lhs = Qst_c[:, cch, :]                       # (64, 32)
                    nc.tensor.matmul(out=zp[0:32, i, :], lhsT=lhs, rhs=ghR2_s,
                                     start=True, stop=True)
                    nc.tensor.matmul(out=zp[32:64, i, :], lhsT=lhs, rhs=ghI2_s,
                                     start=True, stop=True, tile_position=(0, 32))
                nc.scalar.copy(
                    out=Zh2[:, :, grp * 4:(grp + 1) * 4, :],
                    in_=zp.rearrange("p c (q l) -> p q c l", q=4))

    # ---------------- S3 + S4: main loop ----------------
    main = ctx.enter_context(tc.tile_pool(name="main", bufs=1))
    xb16 = main.tile([128, FQ + 2 * HALO], FP16, tag="xb16")
    zbuf = main.tile([128, FQ], F32, tag="zbuf")
    szc = main.tile([128, 2 * NFC], F32, tag="szc")
    sqc = main.tile([128, NFC], F32, tag="sqc")

    nc.vector.memset(xb16[0:OC, 0:HALO], 0.0)
    nc.vector.memset(xb16[96:128, HALO + FQ:], 0.0)
    for q in range(NQ):
        nc.sync.dma_start(out=xb16[32 * q:32 * q + 32, HALO:HALO + FQ],
                          in_=xoh_f[:, q * FQ:(q + 1) * FQ])
        if q > 0:
            nc.sync.dma_start(out=xb16[32 * q:32 * q + 32, 0:HALO],
                              in_=xoh_f[:, q * FQ - HALO:q * FQ])
        if q < NQ - 1:
            nc.sync.dma_start(out=xb16[32 * q:32 * q + 32, HALO + FQ:],
                              in_=xoh_f[:, (q + 1) * FQ:(q + 1) * FQ + HALO])

    with tc.tile_pool(name="xqp", bufs=3) as xqp, \
         tc.tile_pool(name="h1sp", bufs=2) as h1sp, \
         tc.tile_pool(name="chain", bufs=2) as chain, \
         tc.tile_pool(name="ps_zp", bufs=3, space="PSUM") as ps_zp, \
         tc.tile_pool(name="ps_h1", bufs=1, space="PSUM") as ps_h1:
        for fc in range(NFC):
            f0 = fc * FC
            ZPa = ps_zp.tile([128, 1024], F32, tag="ZP")
            ZPb = ps_zp.tile([128, 1024], F32, tag="ZP")
            ZPh = [ZPa, ZPb]
            xqp2 = []
            for qp in range(2):
                xq = xqp.tile([128, FC], FP16, tag="xq")
                for j in range(2):
                    q = qp * 2 + j
                    nc.sync.dma_start(out=xq[64 * j:64 * j + 64, :],
                                      in_=xbh_f[:, q * FQ + f0:q * FQ + f0 + FC])
                xqp2.append(xq)
            xqs = [xqp2[q // 2][64 * (q % 2):64 * (q % 2) + 64, :] for q in range(NQ)]
            # mlp1 (same weights back-to-back) + gelu
            h1ss = []
            for q in range(NQ):
                h1s = h1sp.tile([128, FC], FP16, tag="h1s")
                j = q % 2
                for s in range(0, FC, 1024):
                    hp = ps_h1.tile([128, 1024], F32, tag="h1p")
                    for s2 in range(0, 1024, 512):
                        nc.tensor.matmul(
                            out=hp[:, s2:s2 + 512],
                            lhsT=wm1T2_s[64 * j:64 * j + 64, :],
                            rhs=xqs[q][:, s + s2:s + s2 + 512],
                            start=True, stop=True, tile_position=(64 * j, 0))
                    nc.scalar.activation(out=h1s[:, s:s + 1024], in_=hp,
                                         func=AF.Gelu, bias=bm1_s, scale=1.0)
                h1ss.append(h1s)
            # local: q-paired block-diag (K=128, M=64)
            for qp in range(2):
                tp = (0, 64 * qp) if qp > 0 else None
                for s in range(0, FC, 512):
                    zsl = ZPh[s // 1024][64 * qp:64 * qp + 64, s % 1024:s % 1024 + 512]
                    nc.tensor.matmul(out=zsl, lhsT=wlocT2_s,
                                     rhs=xqp2[qp][:, s:s + 512],
                                     start=True, stop=False, tile_position=tp,
                                     skip_group_check=True)
            # W-inverse y
            for i in range(NFC):
                h0 = fc * 8 + i
                lhs = Zh2[:, :, :, h0]
                zsl = ZPh[i // 4][:, (i % 4) * 256:(i % 4) * 256 + 256]
                nc.tensor.matmul(out=zsl, lhsT=lhs, rhs=gw2_s,
                                 start=False, stop=False, skip_group_check=True)
            # mlp2 (same weights; last writer -> stop)
            for q in range(NQ):
                tp = (0, 32 * q) if q > 0 else None
                for s in range(0, FC, 512):
                    zsl = ZPh[s // 1024][32 * q:32 * q + 32, s % 1024:s % 1024 + 512]
                    nc.tensor.matmul(out=zsl, lhsT=wm2T_s, rhs=h1ss[q][:, s:s + 512],
                                     start=False, stop=True, tile_position=tp,
                                     skip_group_check=True)

            # --- z-chain ---
            accD = chain.tile([128, FC], FP16, tag="accD")
            for t, (dy, dx) in enumerate(CHAIN_TAPS):
                d = dy * 256 + dx
                xs = xb16[:, HALO + f0 + d:HALO + f0 + d + FC]
                kt = ktaps_s[:, t:t + 1]
                first = t == 0
                nc.vector.scalar_tensor_tensor(out=accD, in0=xs, scalar=kt,
                                               in1=xs if first else accD,
                                               op0=AX.mult,
                                               op1=AX.bypass if first else AX.add)
            accDv = accD.rearrange("p (r w) -> p r w", w=256)
            for t, (dy, dx) in enumerate(CORR_TAPS):
                d = dy * 256 + dx
                col = 0 if dx == -1 else 255
                xsv = xb16[:, HALO + f0 + d:HALO + f0 + d + FC].rearrange(
                    "p (r w) -> p r w", w=256)[:, :, col:col + 1]
                av = accDv[:, :, col:col + 1]
                nc.vector.scalar_tensor_tensor(out=av, in0=xsv,
                                               scalar=kcorr_s[:, t:t + 1],
                                               in1=av, op0=AX.mult, op1=AX.add)
            for hlf in range(2):
                nc.vector.scalar_tensor_tensor(
                    out=zbuf[:, f0 + hlf * 1024:f0 + hlf * 1024 + 1024],
                    in0=ZPh[hlf], scalar=bconst_s,
                    in1=accD[:, hlf * 1024:hlf * 1024 + 1024],
                    op0=AX.add, op1=AX.add,
                    accum_out=szc[:, 2 * fc + hlf:2 * fc + hlf + 1])
            nc.scalar.activation(out=accD, in_=zbuf[:, f0:f0 + FC], func=AF.Square,
                                 accum_out=sqc[:, fc:fc + 1])

    # ---------------- S5: stats ----------------
    st = ctx.enter_context(tc.tile_pool(name="stats", bufs=1))
    with tc.tile_pool(name="ps_st", bufs=1, space="PSUM") as ps_st:
        sums = st.tile([128, 2], F32, tag="sums")
        nc.vector.tensor_reduce(out=sums[:, 0:1], in_=szc,
                                axis=mybir.AxisListType.X, op=AX.add)
        nc.vector.tensor_reduce(out=sums[:, 1:2], in_=sqc,
                                axis=mybir.AxisListType.X, op=AX.add)
        sp = ps_st.tile([32, 2], F32, tag="sp")
        nc.tensor.matmul(out=sp, lhsT=qones_s, rhs=sums, start=True, stop=True)
        mu = st.tile([32, 1], F32, tag="mu")
        negmu = st.tile([32, 1], F32, tag="negmu")
        ex2 = st.tile([32, 1], F32, tag="ex2")
        var = st.tile([32, 1], F32, tag="var")
        s12 = st.tile([32, 2], F32, tag="s12")
        inv_n = 1.0 / float(HW)
        nc.vector.tensor_scalar(out=mu, in0=sp[:, 0:1], scalar1=inv_n,
                                scalar2=None, op0=AX.mult)
        nc.vector.tensor_scalar(out=negmu, in0=sp[:, 0:1], scalar1=-inv_n,
                                scalar2=None, op0=AX.mult)
        nc.vector.tensor_scalar(out=ex2, in0=sp[:, 1:2], scalar1=inv_n,
                                scalar2=None, op0=AX.mult)
        nc.vector.scalar_tensor_tensor(out=var, in0=mu, scalar=negmu, in1=ex2,
                                       op0=AX.mult, op1=AX.add)
        epst = st.tile([32, 1], F32, tag="epst")
        nc.vector.memset(epst, 1e-5)
        nc.scalar.activation(out=var, in_=var, func=AF.Sqrt, bias=epst, scale=1.0)
        nc.vector.reciprocal(out=var, in_=var)                   # rstd
        nc.vector.tensor_tensor(out=s12[:, 0:1], in0=var, in1=gam_s, op=AX.mult)
        nc.vector.tensor_scalar(out=negmu, in0=mu, scalar1=-1.0,
                                scalar2=None, op0=AX.mult)
        nc.vector.scalar_tensor_tensor(out=s12[:, 1:2], in0=s12[:, 0:1],
                                       scalar=negmu, in1=bet_s,
                                       op0=AX.mult, op1=AX.add)
        spb = ps_st.tile([128, 2], F32, tag="spb")
        nc.tensor.matmul(out=spb, lhsT=qonesT_s, rhs=s12, start=True, stop=True)
        s12s = st.tile([128, 2], F32, tag="s12s")
        nc.vector.tensor_copy(out=s12s, in_=spb)

    # ---------------- S6: sweep 2 ----------------
    with tc.tile_pool(name="sw2", bufs=3) as sw2, \
         tc.tile_pool(name="sw2x", bufs=3) as sw2x:
        xins = []
        for fc in range(NFC):
            f0 = fc * FC
            xin = sw2x.tile([128, FC], F32, tag="xin")
            for q in range(NQ):
                nc.sync.dma_start(out=xin[32 * q:32 * q + 32, :],
                                  in_=xo_f[:, q * FQ + f0:q * FQ + f0 + FC])
            xins.append(xin)
        for hc in range(2 * NFC):
            f0 = hc * (FC // 2)
            xin = xins[hc // 2][:, (hc % 2) * 1024:(hc % 2) * 1024 + 1024]
            g = sw2.tile([128, FC // 2], F32, tag="g")
            nc.scalar.activation(out=g, in_=zbuf[:, f0:f0 + 1024], func=AF.Gelu,
                                 bias=s12s[:, 1:2], scale=s12s[:, 0:1])
            ob = sw2.tile([128, FC // 2], F32, tag="ob")
            if hc % 2 == 0:
                nc.gpsimd.tensor_tensor(out=ob, in0=g, in1=xin, op=AX.add)
            else:
                nc.vector.tensor_tensor(out=ob, in0=g, in1=xin, op=AX.add)
            for q in range(NQ):
                nc.sync.dma_start(out=out_f[:, q * FQ + f0:q * FQ + f0 + 1024],
                                  in_=ob[32 * q:32 * q + 32, :])


_PROGRAM = None


def kernel(**inputs):
    global _PROGRAM
    in_maps = _per_core_inputs(inputs)
    if _PROGRAM is None:
        _PROGRAM = _build_program()
    res = run_bass_kernel_spmd(_PROGRAM, in_maps, list(range(N_CORES)))
    x = np.asarray(inputs["x"], np.float32)
    out = np.empty_like(x)
    for core in range(N_CORES):
        b, half = core // 2, core % 2
        out[b, half * 32:half * 32 + 32] = res.results[core]["outp"]
    return out
